# revision 41
# baseline (speedup 1.0000x reference)
"""Distributed Trainium2 kernel for nn_AltBlock (dense transformer block).

Sharding: sequence-parallel across 8 cores. Core c owns 256 query tokens of
batch c//4 (quarter (c%4) of the sequence). qkv/proj/mlp run per-core on the
local tokens with replicated weights; attention needs all keys/values of the
batch, obtained with two ~260KB fp8 AllGathers (knT first, then V)
inside each 4-core batch group, triggered as early as possible (k and v
projections run first, q last, so the collectives overlap the q path).

Attention runs transposed: S^T = [k_tokens(part), q_tokens(free)]; head
pairs are interleaved so their K=64 matmuls occupy disjoint PE row groups
and run concurrently. Alibi (+padding mask) is applied MULTIPLICATIVELY:
the host precomputes e_al = exp(alibi - amax + ALOFF) in fp8e4 (masked
keys = exact 0) and the DVE multiplies it into P_exp = exp(S - s - ALOFF
+ 2^PBITS) after the ACT exp. P is fp8e5 so the PV matmul runs in
DoubleRow mode (2x fp8); row-sums come free from an all-ones column
appended to V (PV output row 64), broadcast over 64 partitions with a
K=1 matmul before the reciprocal. proj runs token-major in DoubleRow with
host-interleaved fp8 weights. Warm-keeper matmuls at kernel start and
across the AllGather stall hold the PE HAM clock-gate at full rate.

Precision: wqkv/wproj fp8e4 (host-scaled x16), w2 fp8e3 (x128), w1 bf16,
activations bf16, P fp8e5, OT fp8e4, LN affine folded into consuming
weights on the host. Expected rel err ~9e-3 vs the fp32 reference.
"""

import math
import numpy as np
from contextlib import ExitStack

B, N, C, H = 2, 1024, 1024, 16
D = C // H          # 64
HID = 4 * C         # 4096
NCORES = 8
GROUP = 4           # cores per batch
TLOC = N // GROUP   # 256 local (query) tokens per core
EPS = 1e-5
WSCALE = 16.0       # host multiplies weights by this; kernel divides on evac
PBITS = 10          # P is scaled by 2^PBITS to sit in fp8e5's range
ALOFF = 4.0         # e_al = exp(alibi - amax + ALOFF), fp8e4
PLAG = 8            # heads of S/exp lead before each PV (hides the v AG)

_CACHE = {}


def _build_nc():
    import concourse.bass as bass
    import concourse.tile as tile
    from concourse import bacc, mybir

    f32 = mybir.dt.float32
    bf16 = mybir.dt.bfloat16
    f8 = mybir.dt.float8e4
    f8e5 = mybir.dt.float8e5
    f8e3 = mybir.dt.float8e3
    AF = mybir.ActivationFunctionType
    OP = mybir.AluOpType
    DR = mybir.MatmulPerfMode.DoubleRow
    RSCALE = 1.0 / WSCALE

    nc = bacc.Bacc(None, target_bir_lowering=False)

    x_in = nc.dram_tensor("x_loc", [TLOC, C], f32, kind="ExternalInput")
    # [H, 128, 8, TLOC]: per head one DMA, 2KB contiguous per partition
    alibi_in = nc.dram_tensor("alibi_t", [H, 128, 8, TLOC], f8,
                              kind="ExternalInput")
    # [3(k,v,q), 8, 128, C] so the k slice can stream first
    wqkv_in = nc.dram_tensor("wqkv_t", [3, 8, 128, C], f8, kind="ExternalInput")
    # DoubleRow-interleaved: [j, ki, i, out] = W[256j + 128i + ki, out]
    wproj_in = nc.dram_tensor("wproj_t", [4, 128, 2, C], f8, kind="ExternalInput")
    w1_in = nc.dram_tensor("w1_t", [8, 128, HID], bf16, kind="ExternalInput")
    w2_in = nc.dram_tensor("w2_t", [32, 128, C], f8e3, kind="ExternalInput")
    # f32 consts packed in one tensor: bqkv 0:24, b1 24:56, nbound 56:72,
    # qscale 72:73
    constf_in = nc.dram_tensor("consts_f32", [128, 73], f32, kind="ExternalInput")
    brows_in = nc.dram_tensor("brows", [1, 2 * C], f32, kind="ExternalInput")
    constsb_in = nc.dram_tensor("consts_bf", [128, 384], bf16, kind="ExternalInput")
    out_ext = nc.dram_tensor("out", [TLOC, C], f32, kind="ExternalOutput")

    def bcast_ap(handle):
        ap = handle[:]
        return bass.AP(tensor=ap.tensor, offset=ap.offset,
                       ap=[[0, 128], [1, 2 * C]])

    with ExitStack() as stack:
        stack.enter_context(nc.allow_low_precision(reason="bf16/fp8 compute"))
        tc = stack.enter_context(tile.TileContext(nc))
        pconst = stack.enter_context(tc.tile_pool(name="pconst", bufs=1))
        pdram = stack.enter_context(tc.tile_pool(name="pdram", bufs=1, space="DRAM"))

        # ---- inputs / persistents ----
        constsb_sb = pconst.tile([128, 384], bf16, name="constsb_sb")
        nc.sync.dma_start(constsb_sb, constsb_in[:])
        ident = constsb_sb[:, 0:128]
        ones64 = constsb_sb[:, 128:192]
        sel_64 = constsb_sb[:, 192:194]
        sel2T = constsb_sb[0:2, 194:322]

        constf_sb = pconst.tile([128, 73], f32, name="constf_sb")
        nc.sync.dma_start(constf_sb, constf_in[:])
        bqkv_sb = constf_sb[:, 0:24]
        b1_sb = constf_sb[:, 24:56]
        nbound_sb = constf_sb[:, 56:72]
        lnscale_sb = constf_sb[0:2, 72:73]
        bb_sb = pconst.tile([128, 2 * C], f32, name="bb_sb")
        nc.sync.dma_start(bb_sb, bcast_ap(brows_in))
        bpbc_sb = bb_sb[:, 0:C]
        b2bc_sb = bb_sb[:, C:2 * C]
        eps_sb = pconst.tile([128, 1], f32, name="eps_sb")
        nc.vector.memset(eps_sb, EPS)
        tiny_sb = pconst.tile([128, 1], f32, name="tiny_sb")
        nc.vector.memset(tiny_sb, 1e-24)

        # V staging, [token, tt, head, 65] fp8; col 64 = 1.0 so the PV matmul
        # also produces the softmax row-sum in PSUM partition 64.
        v_loc = pconst.tile([128, 2, H, 65], f8, name="v_loc")
        nc.vector.memset(v_loc[:, :, :, 64:65].rearrange("p a b c -> p (a b c)"),
                         1.0)

        wproj_sb = pconst.tile([128, 4, 2, C], f8, name="wproj_sb")
        w1_sb = pconst.tile([128, 8, HID], bf16, name="w1_sb")

        x1_sb = pconst.tile([128, 2, C], f32, name="x1_sb")
        xb_sb = pconst.tile([128, 2, C], f32, name="xb_sb")
        qnT = pconst.tile([128, 8, TLOC], bf16, name="qnT")
        knT_loc = pconst.tile([128, 8, TLOC], f8, name="knT_loc")
        OT_sb = pconst.tile([128, 8, TLOC], f8, name="OT_sb")

        KNW = 8 * TLOC               # 2048 fp8 cols of knT payload
        VW = 2 * H * 65              # 2080 fp8 cols of V payload
        bounce_kn = pdram.tile([128, KNW], f8, name="bounce_kn")
        ag_kn = pdram.tile([512, KNW], f8, name="ag_kn")
        bounce_v = pdram.tile([128, VW], f8, name="bounce_v")
        ag_v = pdram.tile([512, VW], f8, name="ag_v")

        def layernorm(pool, x_slice, out_t):
            # plain LN (affine is folded into the next matmul's weights)
            stats = pool.tile([128, 2, 6], f32, name="lnstats", tag="lnstats")
            for sg in range(2):
                nc.vector.bn_stats(out=stats[:, sg, :],
                                   in_=x_slice[:, sg * 512:(sg + 1) * 512])
            mv = pool.tile([128, 2], f32, name="lnmv", tag="lnmv")
            nc.vector.bn_aggr(out=mv, in_=stats)
            std = pool.tile([128, 1], f32, name="lnstd", tag="lnstd")
            nc.scalar.activation(out=std, in_=mv[:, 1:2], func=AF.Sqrt,
                                 bias=eps_sb[:, 0:1])
            rstd = pool.tile([128, 1], f32, name="lnrstd", tag="lnrstd")
            nc.vector.reciprocal_approx_fast(out=rstd, in_=std)
            nc.vector.tensor_scalar(out=out_t, in0=x_slice, scalar1=mv[:, 0:1],
                                    scalar2=rstd, op0=OP.subtract, op1=OP.mult)

        # ============== Phase A: LN1, qkv (k,v first), merged AllGather =====
        with tc.tile_pool(name="pA", bufs=1) as pA, \
             tc.tile_pool(name="psA", bufs=1, space="PSUM") as psA, \
             tc.tile_pool(name="ptmpA", bufs=2) as ptmpA:
            # warm-keeper: get HAM to K=8/8 before the real matmuls arrive
            for i in range(120):
                wps = psA.tile([128, 64], f32, name="warm", tag="mm", bufs=4)
                nc.tensor.matmul(wps, lhsT=ident, rhs=ones64)
            x_sb = pA.tile([128, 2, C], f32, name="x_sb")
            nc.sync.dma_start(x_sb[:], x_in.rearrange("(a p) b -> p a b", a=2))
            # wqkv_sb columns: 0-7 = K, 8-15 = V, 16-23 = Q (k streams first)
            wqkv_sb = pA.tile([128, 8, 3 * C], f8, name="wqkv_sb")
            for p in range(3):
                nc.sync.dma_start(
                    wqkv_sb[:, :, p * C:(p + 1) * C],
                    wqkv_in[p].rearrange("a p b -> p a b"))

            h_sb = pA.tile([128, 2, C], bf16, name="h_sb")
            for tt in range(2):
                layernorm(ptmpA, x_sb[:, tt, :], h_sb[:, tt, :])
            hT = pA.tile([128, 8, TLOC], bf16, name="hT")
            for tt in range(2):
                for cp in range(4):
                    tp = psA.tile([128, 2, 128], bf16, name="tp", tag="tp", bufs=2)
                    for k in range(2):
                        cc = 2 * cp + k
                        nc.tensor.transpose(
                            tp[:, k, :], h_sb[:, tt, cc * 128:(cc + 1) * 128], ident)
                    nc.scalar.activation(
                        out=hT[:, 2 * cp:2 * cp + 2, tt * 128:(tt + 1) * 128],
                        in_=tp, func=AF.Copy)

            qkv_sb = pA.tile([128, 24, TLOC], bf16, name="qkv_sb")

            def qkv_super(sup, evac_dve):
                pss = [psA.tile([128, 2, TLOC], f32, name=f"qps{g}", tag="mm",
                                bufs=4) for g in range(2)]
                for blk in range(4):
                    for cc in range(8):
                        nc.tensor.matmul(
                            pss[blk // 2][:, blk % 2, :],
                            lhsT=wqkv_sb[:, cc,
                                         sup * 512 + blk * 128:
                                         sup * 512 + (blk + 1) * 128],
                            rhs=hT[:, cc, :],
                            start=(cc == 0), stop=(cc == 7))
                for blk in range(4):
                    cb = sup * 4 + blk
                    if evac_dve:
                        nc.vector.tensor_scalar(
                            out=qkv_sb[:, cb, :], in0=pss[blk // 2][:, blk % 2, :],
                            scalar1=RSCALE, scalar2=bqkv_sb[:, cb:cb + 1],
                            op0=OP.mult, op1=OP.add)
                    else:
                        nc.scalar.activation(
                            out=qkv_sb[:, cb, :], in_=pss[blk // 2][:, blk % 2, :],
                            func=AF.Identity, bias=bqkv_sb[:, cb:cb + 1],
                            scale=RSCALE)

            def norm_heads(pool, src_col0, dst, with_scale, half=None):
                # half=0/1 processes blocks 0-3 / 4-7 (overlaps the supers)
                b0, nb = (0, 8) if half is None else (4 * half, 4)
                q2 = pool.tile([128, nb, TLOC], bf16, name="q2", tag="q2",
                               bufs=2)
                nc.vector.tensor_mul(
                    q2, qkv_sb[:, src_col0 + b0:src_col0 + b0 + nb, :],
                    qkv_sb[:, src_col0 + b0:src_col0 + b0 + nb, :])
                nrm = pool.tile([2, nb, TLOC], f32, name="nrm", tag="nrm0",
                                bufs=2)
                for g in range(nb // 2):
                    ssq = psA.tile([2, 2, TLOC], f32, name="ssq", tag="nrm",
                                   bufs=2)
                    for k in range(2):
                        nc.tensor.matmul(ssq[:, k, :], lhsT=sel_64,
                                         rhs=q2[:, 2 * g + k, :])
                    nc.scalar.activation(out=nrm[:, 2 * g:2 * g + 2, :],
                                         in_=ssq, func=AF.Sqrt,
                                         bias=tiny_sb[0:2, 0:1])
                rn_all = pool.tile([2, nb, TLOC], f32, name="rn_all", tag="rn",
                                   bufs=2)
                rn_flat = rn_all.rearrange("p a b -> p (a b)")
                nc.vector.reciprocal_approx_fast(
                    out=rn_flat, in_=nrm.rearrange("p a b -> p (a b)"))
                rnr = pool.tile([2, nb, TLOC], bf16, name="rnr", tag="rnr",
                                bufs=2)
                if with_scale:
                    nc.vector.tensor_scalar_mul(
                        out=rnr.rearrange("p a b -> p (a b)"), in0=rn_flat,
                        scalar1=lnscale_sb[:, 0:1])
                else:
                    nc.vector.tensor_copy(rnr.rearrange("p a b -> p (a b)"),
                                          rn_flat)
                for blk in range(nb):
                    bc = psA.tile([128, TLOC], f32, name="bc", tag="nrm", bufs=2)
                    nc.tensor.matmul(bc, lhsT=sel2T, rhs=rnr[:, blk, :])
                    nc.vector.tensor_mul(dst[:, b0 + blk, :], bc,
                                         qkv_sb[:, src_col0 + b0 + blk, :])

            # K first: the kn AllGather is the critical path of attention
            qkv_super(0, evac_dve=False)
            norm_heads(ptmpA, 0, knT_loc, with_scale=False, half=0)
            qkv_super(1, evac_dve=False)
            norm_heads(ptmpA, 0, knT_loc, with_scale=False, half=1)
            nc.sync.dma_start(bounce_kn, knT_loc.rearrange("p a b -> p (a b)"))
            nc.gpsimd.collective_compute(
                "AllGather", OP.bypass,
                ins=[bounce_kn.opt()], outs=[ag_kn.opt()],
                replica_groups=[[0, 1, 2, 3], [4, 5, 6, 7]],
            )
            # V next, transposed into the per-head vext layout
            for sup in (2, 3):
                qkv_super(sup, evac_dve=False)
            for tt in range(2):
                for cb in range(8):
                    tp2 = psA.tile([128, 128], bf16, name="tp2", tag="tp",
                                   bufs=2)
                    nc.tensor.transpose(tp2, qkv_sb[:, 8 + cb,
                                                    tt * 128:(tt + 1) * 128],
                                        ident)
                    nc.vector.tensor_copy(
                        v_loc[:, tt, 2 * cb:2 * cb + 2, 0:64],
                        tp2.rearrange("p (a b) -> p a b", a=2))
            nc.sync.dma_start(bounce_v,
                              v_loc.rearrange("p a b c -> p (a b c)"))
            nc.gpsimd.collective_compute(
                "AllGather", OP.bypass,
                ins=[bounce_v.opt()], outs=[ag_v.opt()],
                replica_groups=[[0, 1, 2, 3], [4, 5, 6, 7]],
            )
            # Q supers + q normalization overlap the collectives
            for sup in (4, 5):
                qkv_super(sup, evac_dve=True)
            norm_heads(ptmpA, 16, qnT, with_scale=True)
            # attn residual base: x + bproj (x_sb dies with phase A)
            for tt in range(2):
                nc.vector.tensor_add(xb_sb[:, tt, :], x_sb[:, tt, :], bpbc_sb)
            # warm-keepers covering the AllGather stall (no data deps)
            for i in range(220):
                wps = psA.tile([128, 64], f32, name="warm2", tag="mm", bufs=4)
                nc.tensor.matmul(wps, lhsT=ident, rhs=ones64)

        # ============== Phase B: attention + proj ==============
        with tc.tile_pool(name="pB", bufs=1) as pB, \
             tc.tile_pool(name="psB", bufs=1, space="PSUM") as psB, \
             tc.tile_pool(name="alst", bufs=8) as alst, \
             tc.tile_pool(name="pexp", bufs=6) as pexp, \
             tc.tile_pool(name="pP", bufs=PLAG + 2) as pP, \
             tc.tile_pool(name="prs", bufs=2) as prs:
            kn_all = pB.tile([128, 4, KNW], f8, name="kn_all")
            nc.sync.dma_start(kn_all,
                              ag_kn.rearrange("(a p) b -> p a b", a=4))

            # alibi tiles stream pairwise with a 2-pair lookahead (issued
            # inside the pass1 loop); first two pairs up front
            al_tiles = [None] * H

            def fetch_al_pair(m):
                al2 = alst.tile([128, 2, 8, TLOC], f8, name="al2", tag="al")
                nc.sync.dma_start(
                    al2, alibi_in[2 * m:2 * m + 2].rearrange("a p b c -> p a b c"))
                al_tiles[2 * m] = al2[:, 0, :, :]
                al_tiles[2 * m + 1] = al2[:, 1, :, :]

            for m in (0, 1, 2):
                fetch_al_pair(m)
            # v readback stalls the sync queue until the v AllGather lands,
            # so it is issued after the early alibi fetches
            v_all = pB.tile([128, 4, 2, H, 65], f8, name="v_all")
            nc.sync.dma_start(v_all.rearrange("p a b c d -> p a (b c d)"),
                              ag_v.rearrange("(a p) b -> p a b", a=4))

            nc.sync.dma_start(wproj_sb[:],
                              wproj_in.rearrange("a p b c -> p a b c"))
            nc.sync.dma_start(w1_sb, w1_in.rearrange("a p b -> p a b"))
            P_tiles = [None] * H
            rows_of = lambda h: slice(64 * (h % 2), 64 * (h % 2) + 64)

            def pass1_pair(m):
                # two heads interleaved: their K=64 matmuls go to disjoint
                # row groups (0-63 / 64-127), so the PE runs them
                # concurrently and pulls LDWEIGHTS ahead
                if m + 3 < 8:
                    fetch_al_pair(m + 3)
                hs = (2 * m, 2 * m + 1)
                Ps = []
                for h in hs:
                    P = pP.tile([128, 8, TLOC], f8e5, name="P", tag="P")
                    P_tiles[h] = P
                    Ps.append(P)
                for g in range(2):
                    Ss = [psB.tile([128, 4, TLOC], f32, name="S", tag="s4",
                                   bufs=2) for _ in range(2)]
                    for j in range(4):
                        b = 4 * g + j
                        r, tt = b // 2, b % 2
                        for i, h in enumerate(hs):
                            rows = rows_of(h)
                            knT_sl = kn_all[rows, r, m * 256 + tt * 128:
                                            m * 256 + tt * 128 + 128]
                            nc.tensor.matmul(Ss[i][:, j, :], lhsT=knT_sl,
                                             rhs=qnT[rows, m, :])
                    for i, h in enumerate(hs):
                        pe = pexp.tile([128, 4, TLOC], bf16, name="pexp",
                                       tag="pexp")
                        nc.scalar.activation(out=pe, in_=Ss[i], func=AF.Exp,
                                             bias=nbound_sb[:, h:h + 1],
                                             scale=1.0)
                        eng = nc.vector if (m + g) % 2 == 0 else nc.gpsimd
                        eng.tensor_mul(
                            Ps[i][:, 4 * g:4 * g + 4, :], pe,
                            al_tiles[h][:, 4 * g:4 * g + 4, :])
                al_tiles[hs[0]] = al_tiles[hs[1]] = None

            def pass2_head(h):
                rows = rows_of(h)
                P = P_tiles[h]
                # one PSUM bank: cols 0:256 hold PV(+rowsum row 64), cols
                # 256:512 hold the broadcast of the row-sum
                pv = psB.tile([65, 2 * TLOC], f32, name="pv", tag="pv", bufs=2)
                for r in range(4):
                    nc.tensor.matmul(pv[:, 0:TLOC], lhsT=v_all[:, r, :, h, :],
                                     rhs=P[:, 2 * r:2 * r + 2, :],
                                     start=(r == 0), stop=(r == 3),
                                     perf_mode=DR)
                rsrow = prs.tile([65, TLOC], bf16, name="rsrow", tag="rsrow")
                nc.vector.tensor_copy(rsrow[64:65, :], pv[64:65, 0:TLOC])
                nc.tensor.matmul(pv[0:64, TLOC:2 * TLOC],
                                 lhsT=ones64[64:65, 0:64],
                                 rhs=rsrow[64:65, :], start=True, stop=True)
                rs = prs.tile([64, TLOC], f32, name="rs", tag="rs")
                nc.vector.reciprocal_approx_fast(out=rs, in_=pv[0:64,
                                                               TLOC:2 * TLOC])
                nc.vector.tensor_mul(OT_sb[rows, h // 2, :], pv[0:64, 0:TLOC],
                                     rs)
                P_tiles[h] = None

            for m in range(8):
                pass1_pair(m)
                if 2 * m >= PLAG:
                    pass2_head(2 * m - PLAG)
                    pass2_head(2 * m - PLAG + 1)
                for i in range(10):
                    jw = psB.tile([128, 64], f32, name="jw", tag="jw", bufs=2)
                    nc.tensor.matmul(jw, lhsT=ident, rhs=ones64)
            for h in range(H - PLAG, H):
                pass2_head(h)

            # proj, token-major DoubleRow: x1 = O @ Wproj / WSCALE + (x + bproj)
            for th in range(2):
                for ch in range(2):
                    pp = psB.tile([128, 512], f32, name="pp", tag="s4", bufs=2)
                    for j in range(4):
                        nc.tensor.matmul(
                            pp, lhsT=OT_sb[:, 2 * j:2 * j + 2,
                                           th * 128:(th + 1) * 128],
                            rhs=wproj_sb[:, j, :, ch * 512:(ch + 1) * 512],
                            start=(j == 0), stop=(j == 3), perf_mode=DR)
                    nc.vector.scalar_tensor_tensor(
                        out=x1_sb[:, th, ch * 512:(ch + 1) * 512],
                        in0=pp, scalar=RSCALE,
                        in1=xb_sb[:, th, ch * 512:(ch + 1) * 512],
                        op0=OP.mult, op1=OP.add)

        # ================= Phase C: LN2 + MLP =================
        with tc.tile_pool(name="pC", bufs=1) as pC, \
             tc.tile_pool(name="psC", bufs=1, space="PSUM") as psC, \
             tc.tile_pool(name="wstC", bufs=3) as wstC, \
             tc.tile_pool(name="ptmpC", bufs=2) as ptmpC:
            out_sb = pC.tile([128, 2, C], f32, name="out_sb")
            y_sb = pC.tile([128, 2, C], bf16, name="y_sb")
            for tt in range(2):
                layernorm(ptmpC, x1_sb[:, tt, :], y_sb[:, tt, :])
            yT = pC.tile([128, 8, TLOC], bf16, name="yT")
            for tt in range(2):
                for cp in range(4):
                    tp4 = psC.tile([128, 2, 128], bf16, name="tp4", tag="tp",
                                   bufs=2)
                    for k in range(2):
                        cc = 2 * cp + k
                        nc.tensor.transpose(
                            tp4[:, k, :], y_sb[:, tt, cc * 128:(cc + 1) * 128],
                            ident)
                    nc.vector.tensor_copy(
                        yT[:, 2 * cp:2 * cp + 2, tt * 128:(tt + 1) * 128],
                        tp4)
            # mlp residual base: x1 + b2 (per-C broadcast)
            x1b_sb = pC.tile([128, 2, C], f32, name="x1b_sb")
            for tt in range(2):
                nc.vector.tensor_add(x1b_sb[:, tt, :], x1_sb[:, tt, :], b2bc_sb)

            h1 = pC.tile([128, 32, TLOC], bf16, name="h1")
            for sup in range(8):
                pss = [psC.tile([128, 2, TLOC], f32, name=f"m1ps{g}", tag="mm",
                                bufs=4) for g in range(2)]
                for blk in range(4):
                    for cc in range(8):
                        nc.tensor.matmul(
                            pss[blk // 2][:, blk % 2, :],
                            lhsT=w1_sb[:, cc,
                                       sup * 512 + blk * 128:
                                       sup * 512 + (blk + 1) * 128],
                            rhs=yT[:, cc, :],
                            start=(cc == 0), stop=(cc == 7))
                for blk in range(4):
                    hb = sup * 4 + blk
                    nc.scalar.activation(out=h1[:, hb, :],
                                         in_=pss[blk // 2][:, blk % 2, :],
                                         func=AF.Gelu,
                                         bias=b1_sb[:, hb:hb + 1], scale=1.0)

            # fc2, token-major, single pass over w2:
            # out = h1 @ W2 / 128 + (x1 + b2)
            pss2 = [psC.tile([128, 512], f32, name=f"m2ps{g}", tag="mm",
                             bufs=4) for g in range(4)]
            for hg in range(8):
                w2t = wstC.tile([128, 4, C], f8e3, name="w2t", tag="w")
                nc.sync.dma_start(
                    w2t, w2_in[4 * hg:4 * hg + 4].rearrange("a p b -> p a b"))
                for k in range(4):
                    hc = 4 * hg + k
                    for th in range(2):
                        for ch in range(2):
                            nc.tensor.matmul(
                                pss2[th * 2 + ch],
                                lhsT=h1[:, hc, th * 128:(th + 1) * 128],
                                rhs=w2t[:, k, ch * 512:(ch + 1) * 512],
                                start=(hc == 0), stop=(hc == 31))
            for th in range(2):
                for ch in range(2):
                    nc.vector.scalar_tensor_tensor(
                        out=out_sb[:, th, ch * 512:(ch + 1) * 512],
                        in0=pss2[th * 2 + ch], scalar=1.0 / 128.0,
                        in1=x1b_sb[:, th, ch * 512:(ch + 1) * 512],
                        op0=OP.mult, op1=OP.add)
            for tt in range(2):
                nc.sync.dma_start(out_ext[tt * 128:(tt + 1) * 128, :],
                                  out_sb[:, tt, :])

    nc.finalize()
    return nc


def _get_nc():
    if "nc" not in _CACHE:
        _CACHE["nc"] = _build_nc()
    return _CACHE["nc"]


def _to_f8(w):
    import ml_dtypes
    return np.clip(w, -240.0, 240.0).astype(ml_dtypes.float8_e4m3fn)


def _dr_interleave(w):
    """[K, M] -> [K//256, 128, 2, M] DoubleRow layout: [j, ki, i, :] =
    w[256j + 128i + ki, :]."""
    K, M = w.shape
    return np.ascontiguousarray(
        w.reshape(K // 256, 2, 128, M).transpose(0, 2, 1, 3))


def _make_in_maps(inputs):
    import ml_dtypes
    bf = ml_dtypes.bfloat16
    x = np.asarray(inputs["x"], np.float32)
    mask = np.asarray(inputs["padding_mask"]).astype(bool)
    alibi = np.asarray(inputs["alibi_bias"], np.float32)
    wqkv = np.asarray(inputs["Wqkv"], np.float32)
    bqkv = np.asarray(inputs["bqkv"], np.float32)
    wproj = np.asarray(inputs["Wproj"], np.float32)
    bproj = np.asarray(inputs["bproj"], np.float32)
    w1 = np.asarray(inputs["W1"], np.float32)
    b1 = np.asarray(inputs["b1"], np.float32)
    w2 = np.asarray(inputs["W2"], np.float32)
    b2 = np.asarray(inputs["b2"], np.float32)
    g1 = np.asarray(inputs["ln1_g"], np.float32)
    bln1 = np.asarray(inputs["ln1_b"], np.float32)
    g2 = np.asarray(inputs["ln2_g"], np.float32)
    bln2 = np.asarray(inputs["ln2_b"], np.float32)
    ls = np.asarray(inputs["logit_scale"], np.float32).reshape(H)
    scale = np.exp(np.minimum(ls, math.log(100.0))).astype(np.float32)
    amax = float(alibi.max())
    # P = exp(S + nbound) * e_al stays within fp8e5 range (max ~2^10 * e)
    nbound = PBITS * math.log(2.0) - scale - ALOFF
    nbound = np.ascontiguousarray(np.tile(nbound.astype(np.float32)[None, :],
                                          (128, 1)))

    # fold LN affine into the consuming weight matrices
    wqkv_eff = g1[:, None] * wqkv
    bqkv_eff = bqkv + bln1 @ wqkv
    w1_eff = g2[:, None] * w1
    b1_eff = b1 + bln2 @ w1

    # reorder qkv -> (k, v, q) so the AllGather inputs compute first
    wq, wk, wv = wqkv_eff[:, 0:C], wqkv_eff[:, C:2 * C], wqkv_eff[:, 2 * C:3 * C]
    bq, bk, bv = bqkv_eff[0:C], bqkv_eff[C:2 * C], bqkv_eff[2 * C:3 * C]
    wqkv_re = np.concatenate([wk, wv, wq], axis=1)          # [C, 3C]
    bqkv_re = np.concatenate([bk, bv, bq])

    consts = np.zeros((128, 384), dtype=np.float32)
    consts[:, 0:128] = np.eye(128, dtype=np.float32)
    consts[:, 128:192] = 1.0
    consts[0:64, 192] = 1.0
    consts[64:128, 193] = 1.0
    consts[0, 194:258] = 1.0
    consts[1, 258:322] = 1.0
    consts = np.ascontiguousarray(consts)

    constf = np.zeros((128, 73), np.float32)
    constf[:, 0:24] = bqkv_re.reshape(24, 128).T
    constf[:, 24:56] = b1_eff.reshape(32, 128).T
    constf[:, 56:72] = nbound
    constf[:, 72] = float(scale[0])
    common = {
        "wqkv_t": _to_f8((WSCALE * wqkv_re).T.reshape(3, C, 8, 128)
                         .transpose(0, 2, 3, 1)),
        "wproj_t": _to_f8(WSCALE * _dr_interleave(wproj)),
        "w1_t": np.ascontiguousarray(w1_eff.reshape(8, 128, HID)).astype(bf),
        "w2_t": np.clip(128.0 * w2.reshape(32, 128, C), -15.5, 15.5)
                .astype(ml_dtypes.float8_e3m4),
        "consts_f32": np.ascontiguousarray(constf),
        "brows": np.ascontiguousarray(
            np.concatenate([bproj, b2]).reshape(1, 2 * C)),
        "consts_bf": consts.astype(bf),
    }
    in_maps = []
    for c in range(NCORES):
        b, qi = divmod(c, GROUP)
        q0 = qi * TLOC
        alT = alibi[b, :, q0:q0 + TLOC, :].transpose(0, 2, 1)  # [H, N(k), TLOC]
        alT = alT + np.where(mask[b], np.float32(-1e9),
                             np.float32(0.0)).astype(np.float32)[None, :, None]
        # multiplicative alibi factor, [H, 128, 8, TLOC] fp8e4,
        # one contiguous DMA per head; masked keys become exactly 0
        alT = np.exp(np.minimum(alT - amax + ALOFF, 5.0))
        alT = np.ascontiguousarray(
            alT.reshape(H, 8, 128, TLOC).transpose(0, 2, 1, 3)) \
            .astype(ml_dtypes.float8_e4m3fn)
        m = dict(common)
        m["x_loc"] = np.ascontiguousarray(x[b, q0:q0 + TLOC, :])
        m["alibi_t"] = alT
        in_maps.append(m)
    return in_maps


def _run(inputs, trace=False):
    from concourse import bass_utils
    nc = _get_nc()
    in_maps = _make_in_maps(inputs)
    res = bass_utils.run_bass_kernel_spmd(
        nc, in_maps, core_ids=list(range(NCORES)), trace=trace)
    outs = [np.asarray(res.results[c]["out"]) for c in range(NCORES)]
    y = np.stack(outs).reshape(B, GROUP * TLOC, C)
    return y.astype(np.float32), res


def kernel(**inputs):
    y, _ = _run(inputs, trace=False)
    return y


# revision 42
# speedup vs baseline: 1.0320x; 1.0320x over previous
"""Distributed Trainium2 kernel for nn_AltBlock (dense transformer block).

Sharding: sequence-parallel across 8 cores. Core c owns 256 query tokens of
batch c//4 (quarter (c%4) of the sequence). qkv/proj/mlp run per-core on the
local tokens with replicated weights; attention needs all keys/values of the
batch, obtained with two ~260KB fp8 AllGathers (knT first, then V)
inside each 4-core batch group, triggered as early as possible (k and v
projections run first, q last, so the collectives overlap the q path).

Attention runs transposed: S^T = [k_tokens(part), q_tokens(free)]; head
pairs are interleaved so their K=64 matmuls occupy disjoint PE row groups
and run concurrently. Alibi (+padding mask) is applied MULTIPLICATIVELY:
the host precomputes e_al = exp(alibi - amax + ALOFF) in fp8e4 (masked
keys = exact 0) and the DVE multiplies it into P_exp = exp(S - s - ALOFF
+ 2^PBITS) after the ACT exp. P is fp8e5 so the PV matmul runs in
DoubleRow mode (2x fp8); row-sums come free from an all-ones column
appended to V (PV output row 64), broadcast over 64 partitions with a
K=1 matmul before the reciprocal. proj runs token-major in DoubleRow with
host-interleaved fp8 weights. Warm-keeper matmuls at kernel start and
across the AllGather stall hold the PE HAM clock-gate at full rate.

Precision: wqkv/wproj fp8e4 (host-scaled x16), w2 fp8e3 (x128), w1 bf16,
activations bf16, P fp8e5, OT fp8e4, LN affine folded into consuming
weights on the host. Expected rel err ~9e-3 vs the fp32 reference.
"""

import math
import numpy as np
from contextlib import ExitStack

B, N, C, H = 2, 1024, 1024, 16
D = C // H          # 64
HID = 4 * C         # 4096
NCORES = 8
GROUP = 4           # cores per batch
TLOC = N // GROUP   # 256 local (query) tokens per core
EPS = 1e-5
WSCALE = 16.0       # host multiplies weights by this; kernel divides on evac
PBITS = 10          # P is scaled by 2^PBITS to sit in fp8e5's range
ALOFF = 4.0         # e_al = exp(alibi - amax + ALOFF), fp8e4
PLAG = 8            # heads of S/exp lead before each PV (hides the v AG)

_CACHE = {}


def _build_nc():
    import concourse.bass as bass
    import concourse.tile as tile
    from concourse import bacc, mybir

    f32 = mybir.dt.float32
    bf16 = mybir.dt.bfloat16
    f8 = mybir.dt.float8e4
    f8e5 = mybir.dt.float8e5
    f8e3 = mybir.dt.float8e3
    AF = mybir.ActivationFunctionType
    OP = mybir.AluOpType
    DR = mybir.MatmulPerfMode.DoubleRow
    RSCALE = 1.0 / WSCALE

    nc = bacc.Bacc(None, target_bir_lowering=False)

    x_in = nc.dram_tensor("x_loc", [TLOC, C], f32, kind="ExternalInput")
    # [H, 128, 8, TLOC]: per head one DMA, 2KB contiguous per partition
    alibi_in = nc.dram_tensor("alibi_t", [H, 128, 8, TLOC], f8,
                              kind="ExternalInput")
    # [3(k,v,q), 8, 128, C] so the k slice can stream first
    wqkv_in = nc.dram_tensor("wqkv_t", [3, 8, 128, C], f8, kind="ExternalInput")
    # DoubleRow-interleaved: [j, ki, i, out] = W[256j + 128i + ki, out]
    wproj_in = nc.dram_tensor("wproj_t", [4, 128, 2, C], f8, kind="ExternalInput")
    w1_in = nc.dram_tensor("w1_t", [8, 128, HID], bf16, kind="ExternalInput")
    w2_in = nc.dram_tensor("w2_t", [32, 128, C], f8e3, kind="ExternalInput")
    # f32 consts packed in one tensor: bqkv 0:24, b1 24:56, nbound 56:72,
    # qscale 72:73
    constf_in = nc.dram_tensor("consts_f32", [128, 73], f32, kind="ExternalInput")
    brows_in = nc.dram_tensor("brows", [1, 2 * C], f32, kind="ExternalInput")
    constsb_in = nc.dram_tensor("consts_bf", [128, 384], bf16, kind="ExternalInput")
    out_ext = nc.dram_tensor("out", [TLOC, C], f32, kind="ExternalOutput")

    def bcast_ap(handle):
        ap = handle[:]
        return bass.AP(tensor=ap.tensor, offset=ap.offset,
                       ap=[[0, 128], [1, 2 * C]])

    with ExitStack() as stack:
        stack.enter_context(nc.allow_low_precision(reason="bf16/fp8 compute"))
        tc = stack.enter_context(tile.TileContext(nc))
        pconst = stack.enter_context(tc.tile_pool(name="pconst", bufs=1))
        pdram = stack.enter_context(tc.tile_pool(name="pdram", bufs=1, space="DRAM"))

        # ---- inputs / persistents ----
        constsb_sb = pconst.tile([128, 384], bf16, name="constsb_sb")
        nc.sync.dma_start(constsb_sb, constsb_in[:])
        ident = constsb_sb[:, 0:128]
        ones64 = constsb_sb[:, 128:192]
        sel_64 = constsb_sb[:, 192:194]
        sel2T = constsb_sb[0:2, 194:322]

        constf_sb = pconst.tile([128, 73], f32, name="constf_sb")
        nc.sync.dma_start(constf_sb, constf_in[:])
        bqkv_sb = constf_sb[:, 0:24]
        b1_sb = constf_sb[:, 24:56]
        nbound_sb = constf_sb[:, 56:72]
        lnscale_sb = constf_sb[0:2, 72:73]
        bb_sb = pconst.tile([128, 2 * C], f32, name="bb_sb")
        nc.sync.dma_start(bb_sb, bcast_ap(brows_in))
        bpbc_sb = bb_sb[:, 0:C]
        b2bc_sb = bb_sb[:, C:2 * C]
        eps_sb = pconst.tile([128, 1], f32, name="eps_sb")
        nc.vector.memset(eps_sb, EPS)
        tiny_sb = pconst.tile([128, 1], f32, name="tiny_sb")
        nc.vector.memset(tiny_sb, 1e-24)

        # V staging, [token, tt, head, 65] fp8; col 64 = 1.0 so the PV matmul
        # also produces the softmax row-sum in PSUM partition 64.
        v_loc = pconst.tile([128, 2, H, 65], f8, name="v_loc")
        nc.vector.memset(v_loc[:, :, :, 64:65].rearrange("p a b c -> p (a b c)"),
                         1.0)

        wproj_sb = pconst.tile([128, 4, 2, C], f8, name="wproj_sb")
        w1_sb = pconst.tile([128, 8, HID], bf16, name="w1_sb")

        x1_sb = pconst.tile([128, 2, C], f32, name="x1_sb")
        xb_sb = pconst.tile([128, 2, C], f32, name="xb_sb")
        qnT = pconst.tile([128, 8, TLOC], bf16, name="qnT")
        knT_loc = pconst.tile([128, 8, TLOC], f8, name="knT_loc")
        OT_sb = pconst.tile([128, 8, TLOC], f8, name="OT_sb")

        KNW = 8 * TLOC               # 2048 fp8 cols of knT payload
        VW = 2 * H * 65              # 2080 fp8 cols of V payload
        bounce_kn = pdram.tile([128, KNW], f8, name="bounce_kn")
        ag_kn = pdram.tile([512, KNW], f8, name="ag_kn")
        bounce_v = pdram.tile([128, VW], f8, name="bounce_v")
        ag_v = pdram.tile([512, VW], f8, name="ag_v")

        def layernorm(pool, x_slice, out_t):
            # plain LN (affine is folded into the next matmul's weights)
            stats = pool.tile([128, 2, 6], f32, name="lnstats", tag="lnstats")
            for sg in range(2):
                nc.vector.bn_stats(out=stats[:, sg, :],
                                   in_=x_slice[:, sg * 512:(sg + 1) * 512])
            mv = pool.tile([128, 2], f32, name="lnmv", tag="lnmv")
            nc.vector.bn_aggr(out=mv, in_=stats)
            std = pool.tile([128, 1], f32, name="lnstd", tag="lnstd")
            nc.scalar.activation(out=std, in_=mv[:, 1:2], func=AF.Sqrt,
                                 bias=eps_sb[:, 0:1])
            rstd = pool.tile([128, 1], f32, name="lnrstd", tag="lnrstd")
            nc.vector.reciprocal_approx_fast(out=rstd, in_=std)
            nc.vector.tensor_scalar(out=out_t, in0=x_slice, scalar1=mv[:, 0:1],
                                    scalar2=rstd, op0=OP.subtract, op1=OP.mult)

        # ============== Phase A: LN1, qkv (k,v first), merged AllGather =====
        with tc.tile_pool(name="pA", bufs=1) as pA, \
             tc.tile_pool(name="psA", bufs=1, space="PSUM") as psA, \
             tc.tile_pool(name="ptmpA", bufs=2) as ptmpA:
            # warm-keeper: get HAM to K=8/8 before the real matmuls arrive
            for i in range(120):
                wps = psA.tile([128, 64], f32, name="warm", tag="mm", bufs=4)
                nc.tensor.matmul(wps, lhsT=ident, rhs=ones64)
            x_sb = pA.tile([128, 2, C], f32, name="x_sb")
            nc.sync.dma_start(x_sb[:], x_in.rearrange("(a p) b -> p a b", a=2))
            # wqkv_sb columns: 0-7 = K, 8-15 = V, 16-23 = Q (k streams first)
            wqkv_sb = pA.tile([128, 8, 3 * C], f8, name="wqkv_sb")
            for p in range(3):
                nc.sync.dma_start(
                    wqkv_sb[:, :, p * C:(p + 1) * C],
                    wqkv_in[p].rearrange("a p b -> p a b"))

            h_sb = pA.tile([128, 2, C], bf16, name="h_sb")
            for tt in range(2):
                layernorm(ptmpA, x_sb[:, tt, :], h_sb[:, tt, :])
            hT = pA.tile([128, 8, TLOC], bf16, name="hT")
            for tt in range(2):
                for cp in range(4):
                    tp = psA.tile([128, 2, 128], bf16, name="tp", tag="tp", bufs=2)
                    for k in range(2):
                        cc = 2 * cp + k
                        nc.tensor.transpose(
                            tp[:, k, :], h_sb[:, tt, cc * 128:(cc + 1) * 128], ident)
                    nc.scalar.activation(
                        out=hT[:, 2 * cp:2 * cp + 2, tt * 128:(tt + 1) * 128],
                        in_=tp, func=AF.Copy)

            qkv_sb = pA.tile([128, 24, TLOC], bf16, name="qkv_sb")

            def qkv_super(sup, evac_dve):
                pss = [psA.tile([128, 2, TLOC], f32, name=f"qps{g}", tag="mm",
                                bufs=4) for g in range(2)]
                for blk in range(4):
                    for cc in range(8):
                        nc.tensor.matmul(
                            pss[blk // 2][:, blk % 2, :],
                            lhsT=wqkv_sb[:, cc,
                                         sup * 512 + blk * 128:
                                         sup * 512 + (blk + 1) * 128],
                            rhs=hT[:, cc, :],
                            start=(cc == 0), stop=(cc == 7))
                for blk in range(4):
                    cb = sup * 4 + blk
                    if evac_dve:
                        nc.vector.tensor_scalar(
                            out=qkv_sb[:, cb, :], in0=pss[blk // 2][:, blk % 2, :],
                            scalar1=RSCALE, scalar2=bqkv_sb[:, cb:cb + 1],
                            op0=OP.mult, op1=OP.add)
                    else:
                        nc.scalar.activation(
                            out=qkv_sb[:, cb, :], in_=pss[blk // 2][:, blk % 2, :],
                            func=AF.Identity, bias=bqkv_sb[:, cb:cb + 1],
                            scale=RSCALE)

            def norm_heads(pool, src_col0, dst, with_scale, half=None):
                # half=0/1 processes blocks 0-3 / 4-7 (overlaps the supers)
                b0, nb = (0, 8) if half is None else (4 * half, 4)
                q2 = pool.tile([128, nb, TLOC], bf16, name="q2", tag="q2",
                               bufs=2)
                nc.vector.tensor_mul(
                    q2, qkv_sb[:, src_col0 + b0:src_col0 + b0 + nb, :],
                    qkv_sb[:, src_col0 + b0:src_col0 + b0 + nb, :])
                nrm = pool.tile([2, nb, TLOC], f32, name="nrm", tag="nrm0",
                                bufs=2)
                for g in range(nb // 2):
                    ssq = psA.tile([2, 2, TLOC], f32, name="ssq", tag="nrm",
                                   bufs=2)
                    for k in range(2):
                        nc.tensor.matmul(ssq[:, k, :], lhsT=sel_64,
                                         rhs=q2[:, 2 * g + k, :])
                    nc.scalar.activation(out=nrm[:, 2 * g:2 * g + 2, :],
                                         in_=ssq, func=AF.Sqrt,
                                         bias=tiny_sb[0:2, 0:1])
                rn_all = pool.tile([2, nb, TLOC], f32, name="rn_all", tag="rn",
                                   bufs=2)
                rn_flat = rn_all.rearrange("p a b -> p (a b)")
                nc.vector.reciprocal_approx_fast(
                    out=rn_flat, in_=nrm.rearrange("p a b -> p (a b)"))
                rnr = pool.tile([2, nb, TLOC], bf16, name="rnr", tag="rnr",
                                bufs=2)
                if with_scale:
                    nc.vector.tensor_scalar_mul(
                        out=rnr.rearrange("p a b -> p (a b)"), in0=rn_flat,
                        scalar1=lnscale_sb[:, 0:1])
                else:
                    nc.vector.tensor_copy(rnr.rearrange("p a b -> p (a b)"),
                                          rn_flat)
                for blk in range(nb):
                    bc = psA.tile([128, TLOC], f32, name="bc", tag="nrm", bufs=2)
                    nc.tensor.matmul(bc, lhsT=sel2T, rhs=rnr[:, blk, :])
                    nc.vector.tensor_mul(dst[:, b0 + blk, :], bc,
                                         qkv_sb[:, src_col0 + b0 + blk, :])

            # K first: the kn AllGather is the critical path of attention
            qkv_super(0, evac_dve=True)
            norm_heads(ptmpA, 0, knT_loc, with_scale=False, half=0)
            qkv_super(1, evac_dve=True)
            norm_heads(ptmpA, 0, knT_loc, with_scale=False, half=1)
            nc.sync.dma_start(bounce_kn, knT_loc.rearrange("p a b -> p (a b)"))
            nc.gpsimd.collective_compute(
                "AllGather", OP.bypass,
                ins=[bounce_kn.opt()], outs=[ag_kn.opt()],
                replica_groups=[[0, 1, 2, 3], [4, 5, 6, 7]],
            )
            # V next, transposed into the per-head vext layout
            for sup in (2, 3):
                qkv_super(sup, evac_dve=False)
            for tt in range(2):
                for cb in range(8):
                    tp2 = psA.tile([128, 128], bf16, name="tp2", tag="tp",
                                   bufs=2)
                    nc.tensor.transpose(tp2, qkv_sb[:, 8 + cb,
                                                    tt * 128:(tt + 1) * 128],
                                        ident)
                    nc.vector.tensor_copy(
                        v_loc[:, tt, 2 * cb:2 * cb + 2, 0:64],
                        tp2.rearrange("p (a b) -> p a b", a=2))
            nc.sync.dma_start(bounce_v,
                              v_loc.rearrange("p a b c -> p (a b c)"))
            nc.gpsimd.collective_compute(
                "AllGather", OP.bypass,
                ins=[bounce_v.opt()], outs=[ag_v.opt()],
                replica_groups=[[0, 1, 2, 3], [4, 5, 6, 7]],
            )
            # Q supers + q normalization overlap the collectives
            for sup in (4, 5):
                qkv_super(sup, evac_dve=True)
            norm_heads(ptmpA, 16, qnT, with_scale=True)
            # attn residual base: x + bproj (x_sb dies with phase A)
            for tt in range(2):
                nc.vector.tensor_add(xb_sb[:, tt, :], x_sb[:, tt, :], bpbc_sb)
            # warm-keepers covering the AllGather stall (no data deps)
            for i in range(140):
                wps = psA.tile([128, 64], f32, name="warm2", tag="mm", bufs=4)
                nc.tensor.matmul(wps, lhsT=ident, rhs=ones64)

        # ============== Phase B: attention + proj ==============
        with tc.tile_pool(name="pB", bufs=1) as pB, \
             tc.tile_pool(name="psB", bufs=1, space="PSUM") as psB, \
             tc.tile_pool(name="alst", bufs=8) as alst, \
             tc.tile_pool(name="pexp", bufs=6) as pexp, \
             tc.tile_pool(name="pP", bufs=PLAG + 2) as pP, \
             tc.tile_pool(name="prs", bufs=2) as prs:
            kn_all = pB.tile([128, 4, KNW], f8, name="kn_all")
            nc.sync.dma_start(kn_all,
                              ag_kn.rearrange("(a p) b -> p a b", a=4))

            # alibi tiles stream pairwise with a 2-pair lookahead (issued
            # inside the pass1 loop); first two pairs up front
            al_tiles = [None] * H

            def fetch_al_pair(m):
                al2 = alst.tile([128, 2, 8, TLOC], f8, name="al2", tag="al")
                nc.sync.dma_start(
                    al2, alibi_in[2 * m:2 * m + 2].rearrange("a p b c -> p a b c"))
                al_tiles[2 * m] = al2[:, 0, :, :]
                al_tiles[2 * m + 1] = al2[:, 1, :, :]

            for m in (0, 1, 2):
                fetch_al_pair(m)
            # v readback stalls the sync queue until the v AllGather lands,
            # so it is issued after the early alibi fetches
            v_all = pB.tile([128, 4, 2, H, 65], f8, name="v_all")
            nc.sync.dma_start(v_all.rearrange("p a b c d -> p a (b c d)"),
                              ag_v.rearrange("(a p) b -> p a b", a=4))

            nc.sync.dma_start(wproj_sb[:],
                              wproj_in.rearrange("a p b c -> p a b c"))
            nc.sync.dma_start(w1_sb, w1_in.rearrange("a p b -> p a b"))
            P_tiles = [None] * H
            rows_of = lambda h: slice(64 * (h % 2), 64 * (h % 2) + 64)

            def pass1_pair(m):
                # two heads interleaved: their K=64 matmuls go to disjoint
                # row groups (0-63 / 64-127), so the PE runs them
                # concurrently and pulls LDWEIGHTS ahead
                if m + 3 < 8:
                    fetch_al_pair(m + 3)
                hs = (2 * m, 2 * m + 1)
                Ps = []
                for h in hs:
                    P = pP.tile([128, 8, TLOC], f8e5, name="P", tag="P")
                    P_tiles[h] = P
                    Ps.append(P)
                for g in range(2):
                    Ss = [psB.tile([128, 4, TLOC], f32, name="S", tag="s4",
                                   bufs=2) for _ in range(2)]
                    for j in range(4):
                        b = 4 * g + j
                        r, tt = b // 2, b % 2
                        for i, h in enumerate(hs):
                            rows = rows_of(h)
                            knT_sl = kn_all[rows, r, m * 256 + tt * 128:
                                            m * 256 + tt * 128 + 128]
                            nc.tensor.matmul(Ss[i][:, j, :], lhsT=knT_sl,
                                             rhs=qnT[rows, m, :])
                    for i, h in enumerate(hs):
                        pe = pexp.tile([128, 4, TLOC], bf16, name="pexp",
                                       tag="pexp")
                        nc.scalar.activation(out=pe, in_=Ss[i], func=AF.Exp,
                                             bias=nbound_sb[:, h:h + 1],
                                             scale=1.0)
                        eng = nc.vector if (m + g) % 2 == 0 else nc.gpsimd
                        eng.tensor_mul(
                            Ps[i][:, 4 * g:4 * g + 4, :], pe,
                            al_tiles[h][:, 4 * g:4 * g + 4, :])
                al_tiles[hs[0]] = al_tiles[hs[1]] = None

            def pass2_head(h):
                rows = rows_of(h)
                P = P_tiles[h]
                pv = psB.tile([65, TLOC], f32, name="pv", tag="pv", bufs=2)
                for r in range(4):
                    nc.tensor.matmul(pv, lhsT=v_all[:, r, :, h, :],
                                     rhs=P[:, 2 * r:2 * r + 2, :],
                                     start=(r == 0), stop=(r == 3),
                                     perf_mode=DR)
                # row 64 = sum(P); broadcast it to 64 partitions via a K=1
                # matmul, then reciprocal + scale the PV rows
                rsrow = prs.tile([65, TLOC], bf16, name="rsrow", tag="rsrow")
                nc.vector.tensor_copy(rsrow[64:65, :], pv[64:65, :])
                bcp = psB.tile([64, TLOC], f32, name="bcp", tag="bcp", bufs=2)
                nc.tensor.matmul(bcp, lhsT=ones64[64:65, 0:64],
                                 rhs=rsrow[64:65, :])
                rs = prs.tile([64, TLOC], f32, name="rs", tag="rs")
                nc.vector.reciprocal_approx_fast(out=rs, in_=bcp)
                nc.vector.tensor_mul(OT_sb[rows, h // 2, :], pv[0:64, :], rs)
                P_tiles[h] = None

            for m in range(8):
                pass1_pair(m)
                if 2 * m >= PLAG:
                    pass2_head(2 * m - PLAG)
                    pass2_head(2 * m - PLAG + 1)
            for h in range(H - PLAG, H):
                pass2_head(h)

            # proj, token-major DoubleRow: x1 = O @ Wproj / WSCALE + (x + bproj)
            for th in range(2):
                for ch in range(2):
                    pp = psB.tile([128, 512], f32, name="pp", tag="s4", bufs=2)
                    for j in range(4):
                        nc.tensor.matmul(
                            pp, lhsT=OT_sb[:, 2 * j:2 * j + 2,
                                           th * 128:(th + 1) * 128],
                            rhs=wproj_sb[:, j, :, ch * 512:(ch + 1) * 512],
                            start=(j == 0), stop=(j == 3), perf_mode=DR)
                    nc.vector.scalar_tensor_tensor(
                        out=x1_sb[:, th, ch * 512:(ch + 1) * 512],
                        in0=pp, scalar=RSCALE,
                        in1=xb_sb[:, th, ch * 512:(ch + 1) * 512],
                        op0=OP.mult, op1=OP.add)

        # ================= Phase C: LN2 + MLP =================
        with tc.tile_pool(name="pC", bufs=1) as pC, \
             tc.tile_pool(name="psC", bufs=1, space="PSUM") as psC, \
             tc.tile_pool(name="wstC", bufs=3) as wstC, \
             tc.tile_pool(name="ptmpC", bufs=2) as ptmpC:
            out_sb = pC.tile([128, 2, C], f32, name="out_sb")
            y_sb = pC.tile([128, 2, C], bf16, name="y_sb")
            for tt in range(2):
                layernorm(ptmpC, x1_sb[:, tt, :], y_sb[:, tt, :])
            yT = pC.tile([128, 8, TLOC], bf16, name="yT")
            for tt in range(2):
                for cp in range(4):
                    tp4 = psC.tile([128, 2, 128], bf16, name="tp4", tag="tp",
                                   bufs=2)
                    for k in range(2):
                        cc = 2 * cp + k
                        nc.tensor.transpose(
                            tp4[:, k, :], y_sb[:, tt, cc * 128:(cc + 1) * 128],
                            ident)
                    nc.vector.tensor_copy(
                        yT[:, 2 * cp:2 * cp + 2, tt * 128:(tt + 1) * 128],
                        tp4)
            # mlp residual base: x1 + b2 (per-C broadcast)
            x1b_sb = pC.tile([128, 2, C], f32, name="x1b_sb")
            for tt in range(2):
                nc.vector.tensor_add(x1b_sb[:, tt, :], x1_sb[:, tt, :], b2bc_sb)

            h1 = pC.tile([128, 32, TLOC], bf16, name="h1")
            for sup in range(8):
                pss = [psC.tile([128, 2, TLOC], f32, name=f"m1ps{g}", tag="mm",
                                bufs=4) for g in range(2)]
                for blk in range(4):
                    for cc in range(8):
                        nc.tensor.matmul(
                            pss[blk // 2][:, blk % 2, :],
                            lhsT=w1_sb[:, cc,
                                       sup * 512 + blk * 128:
                                       sup * 512 + (blk + 1) * 128],
                            rhs=yT[:, cc, :],
                            start=(cc == 0), stop=(cc == 7))
                for blk in range(4):
                    hb = sup * 4 + blk
                    nc.scalar.activation(out=h1[:, hb, :],
                                         in_=pss[blk // 2][:, blk % 2, :],
                                         func=AF.Gelu,
                                         bias=b1_sb[:, hb:hb + 1], scale=1.0)

            # fc2, token-major, single pass over w2:
            # out = h1 @ W2 / 128 + (x1 + b2)
            pss2 = [psC.tile([128, 512], f32, name=f"m2ps{g}", tag="mm",
                             bufs=4) for g in range(4)]
            for hg in range(8):
                w2t = wstC.tile([128, 4, C], f8e3, name="w2t", tag="w")
                nc.sync.dma_start(
                    w2t, w2_in[4 * hg:4 * hg + 4].rearrange("a p b -> p a b"))
                for k in range(4):
                    hc = 4 * hg + k
                    for th in range(2):
                        for ch in range(2):
                            nc.tensor.matmul(
                                pss2[th * 2 + ch],
                                lhsT=h1[:, hc, th * 128:(th + 1) * 128],
                                rhs=w2t[:, k, ch * 512:(ch + 1) * 512],
                                start=(hc == 0), stop=(hc == 31))
            for th in range(2):
                for ch in range(2):
                    nc.vector.scalar_tensor_tensor(
                        out=out_sb[:, th, ch * 512:(ch + 1) * 512],
                        in0=pss2[th * 2 + ch], scalar=1.0 / 128.0,
                        in1=x1b_sb[:, th, ch * 512:(ch + 1) * 512],
                        op0=OP.mult, op1=OP.add)
            for tt in range(2):
                nc.sync.dma_start(out_ext[tt * 128:(tt + 1) * 128, :],
                                  out_sb[:, tt, :])

    nc.finalize()
    return nc


def _get_nc():
    if "nc" not in _CACHE:
        _CACHE["nc"] = _build_nc()
    return _CACHE["nc"]


def _to_f8(w):
    import ml_dtypes
    return np.clip(w, -240.0, 240.0).astype(ml_dtypes.float8_e4m3fn)


def _dr_interleave(w):
    """[K, M] -> [K//256, 128, 2, M] DoubleRow layout: [j, ki, i, :] =
    w[256j + 128i + ki, :]."""
    K, M = w.shape
    return np.ascontiguousarray(
        w.reshape(K // 256, 2, 128, M).transpose(0, 2, 1, 3))


def _make_in_maps(inputs):
    import ml_dtypes
    bf = ml_dtypes.bfloat16
    x = np.asarray(inputs["x"], np.float32)
    mask = np.asarray(inputs["padding_mask"]).astype(bool)
    alibi = np.asarray(inputs["alibi_bias"], np.float32)
    wqkv = np.asarray(inputs["Wqkv"], np.float32)
    bqkv = np.asarray(inputs["bqkv"], np.float32)
    wproj = np.asarray(inputs["Wproj"], np.float32)
    bproj = np.asarray(inputs["bproj"], np.float32)
    w1 = np.asarray(inputs["W1"], np.float32)
    b1 = np.asarray(inputs["b1"], np.float32)
    w2 = np.asarray(inputs["W2"], np.float32)
    b2 = np.asarray(inputs["b2"], np.float32)
    g1 = np.asarray(inputs["ln1_g"], np.float32)
    bln1 = np.asarray(inputs["ln1_b"], np.float32)
    g2 = np.asarray(inputs["ln2_g"], np.float32)
    bln2 = np.asarray(inputs["ln2_b"], np.float32)
    ls = np.asarray(inputs["logit_scale"], np.float32).reshape(H)
    scale = np.exp(np.minimum(ls, math.log(100.0))).astype(np.float32)
    amax = float(alibi.max())
    # P = exp(S + nbound) * e_al stays within fp8e5 range (max ~2^10 * e)
    nbound = PBITS * math.log(2.0) - scale - ALOFF
    nbound = np.ascontiguousarray(np.tile(nbound.astype(np.float32)[None, :],
                                          (128, 1)))

    # fold LN affine into the consuming weight matrices
    wqkv_eff = g1[:, None] * wqkv
    bqkv_eff = bqkv + bln1 @ wqkv
    w1_eff = g2[:, None] * w1
    b1_eff = b1 + bln2 @ w1

    # reorder qkv -> (k, v, q) so the AllGather inputs compute first
    wq, wk, wv = wqkv_eff[:, 0:C], wqkv_eff[:, C:2 * C], wqkv_eff[:, 2 * C:3 * C]
    bq, bk, bv = bqkv_eff[0:C], bqkv_eff[C:2 * C], bqkv_eff[2 * C:3 * C]
    wqkv_re = np.concatenate([wk, wv, wq], axis=1)          # [C, 3C]
    bqkv_re = np.concatenate([bk, bv, bq])

    consts = np.zeros((128, 384), dtype=np.float32)
    consts[:, 0:128] = np.eye(128, dtype=np.float32)
    consts[:, 128:192] = 1.0
    consts[0:64, 192] = 1.0
    consts[64:128, 193] = 1.0
    consts[0, 194:258] = 1.0
    consts[1, 258:322] = 1.0
    consts = np.ascontiguousarray(consts)

    constf = np.zeros((128, 73), np.float32)
    constf[:, 0:24] = bqkv_re.reshape(24, 128).T
    constf[:, 24:56] = b1_eff.reshape(32, 128).T
    constf[:, 56:72] = nbound
    constf[:, 72] = float(scale[0])
    common = {
        "wqkv_t": _to_f8((WSCALE * wqkv_re).T.reshape(3, C, 8, 128)
                         .transpose(0, 2, 3, 1)),
        "wproj_t": _to_f8(WSCALE * _dr_interleave(wproj)),
        "w1_t": np.ascontiguousarray(w1_eff.reshape(8, 128, HID)).astype(bf),
        "w2_t": np.clip(128.0 * w2.reshape(32, 128, C), -15.5, 15.5)
                .astype(ml_dtypes.float8_e3m4),
        "consts_f32": np.ascontiguousarray(constf),
        "brows": np.ascontiguousarray(
            np.concatenate([bproj, b2]).reshape(1, 2 * C)),
        "consts_bf": consts.astype(bf),
    }
    in_maps = []
    for c in range(NCORES):
        b, qi = divmod(c, GROUP)
        q0 = qi * TLOC
        alT = alibi[b, :, q0:q0 + TLOC, :].transpose(0, 2, 1)  # [H, N(k), TLOC]
        alT = alT + np.where(mask[b], np.float32(-1e9),
                             np.float32(0.0)).astype(np.float32)[None, :, None]
        # multiplicative alibi factor, [H, 128, 8, TLOC] fp8e4,
        # one contiguous DMA per head; masked keys become exactly 0
        alT = np.exp(np.minimum(alT - amax + ALOFF, 5.0))
        alT = np.ascontiguousarray(
            alT.reshape(H, 8, 128, TLOC).transpose(0, 2, 1, 3)) \
            .astype(ml_dtypes.float8_e4m3fn)
        m = dict(common)
        m["x_loc"] = np.ascontiguousarray(x[b, q0:q0 + TLOC, :])
        m["alibi_t"] = alT
        in_maps.append(m)
    return in_maps


def _run(inputs, trace=False):
    from concourse import bass_utils
    nc = _get_nc()
    in_maps = _make_in_maps(inputs)
    res = bass_utils.run_bass_kernel_spmd(
        nc, in_maps, core_ids=list(range(NCORES)), trace=trace)
    outs = [np.asarray(res.results[c]["out"]) for c in range(NCORES)]
    y = np.stack(outs).reshape(B, GROUP * TLOC, C)
    return y.astype(np.float32), res


def kernel(**inputs):
    y, _ = _run(inputs, trace=False)
    return y
